# revision 7
# baseline (speedup 1.0000x reference)
"""AdaptiveBlock Trainium2 kernel, 8-core data-parallel.

Reference computation (per batch b):
    y      = mean(x[b], axis=(H, W))                    # (C,)
    h      = gelu(y @ w1.T)                             # (HIDDEN,)
    y'     = gelu(h @ w2.T)                             # (C,)
    A      = (y' @ wA.T).reshape(H, R)
    Bm     = (y' @ wB.T).reshape(R, W)
    attn   = sigmoid(A @ Bm)                            # (H, W)
    out[b] = broadcast attn over C                      # (C, H, W)

Sharding: B=32 split over 8 cores (4 batches/core), weights replicated.
Memory-bound: each core reads 12.8 MB of x and writes 12.8 MB of output.

Device layout notes:
- y is computed transposed (C on partitions) directly by free-dim reduces.
- All weights are passed pre-transposed (contraction dim on partitions).
- wA additionally has its output columns permuted host-side from i*8+r to
  r*56+i so that the per-batch (8, 56) lhsT of the bilinear matmul can be
  sliced out of contiguous SBUF rows with a cheap SBUF->SBUF DMA.
- The (56, 56) attention map is flattened to (1, 3136), broadcast to all
  128 partitions with a K=1 ones-matmul on the TensorEngine, and sigmoid
  is fused into the PSUM->SBUF copy on the ScalarEngine.
"""

import numpy as np

import concourse.bass as bass
import concourse.tile as tile
from concourse import bacc
from concourse import mybir
from concourse.bass_utils import run_bass_kernel_spmd

F32 = mybir.dt.float32

B, C, H, W = 32, 256, 56, 56
HW = H * W                      # 3136
HIDDEN = 512
RANK = 8
HR = H * RANK                   # 448
NCORES = 8
BLOC = B // NCORES              # 4 batches per core
P = 128
NCC = C // P                    # 2 channel chunks
NHH = HIDDEN // P               # 4 hidden chunks
BCHUNK = 512                    # matmul moving free-dim max


def build_bass(sim_compat: bool = False) -> bacc.Bacc:
    """sim_compat=True swaps exact Gelu (not implemented in CoreSim) for a
    0.5*x stand-in; with |gelu-input| ~ 0.02 this perturbs the final sigmoid
    output by ~1e-5 relative, so the sim still validates all layout/dataflow.
    Hardware builds always use the exact erf-based Gelu."""
    gelu_f = (
        mybir.ActivationFunctionType.Copy
        if sim_compat
        else mybir.ActivationFunctionType.Gelu
    )
    gelu_s = 0.5 if sim_compat else 1.0
    nc = bacc.Bacc(num_devices=NCORES)

    x_d = nc.dram_tensor("x", [BLOC, C, HW], F32, kind="ExternalInput")
    w1t_d = nc.dram_tensor("w1t", [C, HIDDEN], F32, kind="ExternalInput")
    w2t_d = nc.dram_tensor("w2t", [HIDDEN, C], F32, kind="ExternalInput")
    wat_d = nc.dram_tensor("wat", [C, HR], F32, kind="ExternalInput")
    wbt_d = nc.dram_tensor("wbt", [C, HR], F32, kind="ExternalInput")
    out_d = nc.dram_tensor("out", [BLOC, C, HW], F32, kind="ExternalOutput")

    x_v = x_d.ap().rearrange("b (cc p) hw -> b cc p hw", p=P)
    out_v = out_d.ap().rearrange("b (cc p) hw -> b cc p hw", p=P)
    w1t_v = w1t_d.ap().rearrange("(cc p) h -> cc p h", p=P)
    w2t_v = w2t_d.ap().rearrange("(hh p) c -> hh p c", p=P)
    wat_v = wat_d.ap().rearrange("(cc p) n -> cc p n", p=P)
    wbt_v = wbt_d.ap().rearrange("(cc p) n -> cc p n", p=P)

    with tile.TileContext(nc) as tc:
        with (
            tc.tile_pool(name="xin", bufs=4) as xpool,
            tc.tile_pool(name="persist", bufs=1) as ppool,
            tc.tile_pool(name="small", bufs=2) as spool,
            tc.tile_pool(name="bc", bufs=3) as bcpool,
            tc.tile_pool(name="ps_small", bufs=3, space="PSUM") as ps_small,
            tc.tile_pool(name="ps_bc", bufs=4, space="PSUM") as ps_bc,
        ):
            # --- replicated weights into SBUF (contraction dim on partitions)
            w1t = [ppool.tile([P, HIDDEN], F32, tag=f"w1t{cc}", name=f"w1t{cc}") for cc in range(NCC)]
            w2t = [ppool.tile([P, C], F32, tag=f"w2t{hh}", name=f"w2t{hh}") for hh in range(NHH)]
            wat = [ppool.tile([P, HR], F32, tag=f"wat{cc}", name=f"wat{cc}") for cc in range(NCC)]
            wbt = [ppool.tile([P, HR], F32, tag=f"wbt{cc}", name=f"wbt{cc}") for cc in range(NCC)]
            for cc in range(NCC):
                nc.gpsimd.dma_start(w1t[cc][:], w1t_v[cc])
                nc.gpsimd.dma_start(wat[cc][:], wat_v[cc])
                nc.gpsimd.dma_start(wbt[cc][:], wbt_v[cc])
            for hh in range(NHH):
                nc.gpsimd.dma_start(w2t[hh][:], w2t_v[hh])
            ones = ppool.tile([1, P], F32, tag="ones", name="ones")
            nc.vector.memset(ones[:], 1.0)

            # --- stream x in, spatial-sum each (b, channel-chunk) row
            ysum = [ppool.tile([P, BLOC], F32, tag=f"ysum{cc}", name=f"ysum{cc}") for cc in range(NCC)]
            for b in range(BLOC):
                for cc in range(NCC):
                    xt = xpool.tile([P, HW], F32, tag="xt", name="xt")
                    nc.gpsimd.dma_start(xt[:], x_v[b, cc])
                    nc.vector.reduce_sum(
                        ysum[cc][:, b : b + 1], xt[:], axis=mybir.AxisListType.X
                    )

            # --- MLP layer 1: hT[hh] = gelu((y @ w1.T).T chunk), y = ysum/HW
            hT = [ppool.tile([P, BLOC], F32, tag=f"hT{hh}", name=f"hT{hh}") for hh in range(NHH)]
            for hh in range(NHH):
                ph = ps_small.tile([P, BLOC], F32, tag="ps", name="ps")
                for cc in range(NCC):
                    nc.tensor.matmul(
                        ph[:],
                        w1t[cc][:, hh * P : (hh + 1) * P],
                        ysum[cc][:],
                        start=(cc == 0),
                        stop=(cc == NCC - 1),
                    )
                nc.scalar.activation(
                    hT[hh][:], ph[:], gelu_f, scale=gelu_s / HW,
                )

            # --- MLP layer 2: ypT[cc] = gelu((h @ w2.T).T chunk)
            ypT = [ppool.tile([P, BLOC], F32, tag=f"ypT{cc}", name=f"ypT{cc}") for cc in range(NCC)]
            for cc in range(NCC):
                pyp = ps_small.tile([P, BLOC], F32, tag="ps", name="ps")
                for hh in range(NHH):
                    nc.tensor.matmul(
                        pyp[:],
                        w2t[hh][:, cc * P : (cc + 1) * P],
                        hT[hh][:],
                        start=(hh == 0),
                        stop=(hh == NHH - 1),
                    )
                nc.scalar.activation(ypT[cc][:], pyp[:], gelu_f, scale=gelu_s)

            # --- A = y' @ waT (cols r*56+i), Bm = y' @ wbT (cols r*56+j)
            a_sb = ppool.tile([BLOC, HR], F32, tag="a_sb", name="a_sb")
            b_sb = ppool.tile([BLOC, HR], F32, tag="b_sb", name="b_sb")
            for dst, wt in ((a_sb, wat), (b_sb, wbt)):
                pab = ps_small.tile([BLOC, HR], F32, tag="ps", name="ps")
                for cc in range(NCC):
                    nc.tensor.matmul(
                        pab[:],
                        ypT[cc][:],
                        wt[cc][:],
                        start=(cc == 0),
                        stop=(cc == NCC - 1),
                    )
                nc.vector.tensor_copy(dst[:], pab[:])

            # --- per batch: bilinear, sigmoid, broadcast, store
            nchunks = (HW + BCHUNK - 1) // BCHUNK
            for b in range(BLOC):
                ar = spool.tile([RANK, H], F32, tag="ar", name="ar")
                br = spool.tile([RANK, W], F32, tag="br", name="br")
                nc.gpsimd.dma_start(
                    ar[:], a_sb[b : b + 1, :].rearrange("o (r i) -> o r i", r=RANK)
                )
                nc.gpsimd.dma_start(
                    br[:], b_sb[b : b + 1, :].rearrange("o (r j) -> o r j", r=RANK)
                )
                pm = ps_small.tile([H, W], F32, tag="ps", name="ps")
                nc.tensor.matmul(pm[:], ar[:], br[:], start=True, stop=True)
                msb = spool.tile([H, W], F32, tag="msb", name="msb")
                nc.scalar.copy(msb[:], pm[:])
                flat = spool.tile([1, HW], F32, tag="flat", name="flat")
                nc.gpsimd.dma_start(
                    flat[0:1, :].rearrange("o (i j) -> o i j", i=H), msb[:, :]
                )
                bc = bcpool.tile([P, HW], F32, tag="bc", name="bc")
                for k in range(nchunks):
                    off = k * BCHUNK
                    sz = min(BCHUNK, HW - off)
                    pb = ps_bc.tile([P, sz], F32, tag="psbc", name="psbc")
                    nc.tensor.matmul(
                        pb[:], ones[:], flat[0:1, off : off + sz],
                        start=True, stop=True,
                    )
                    nc.scalar.activation(
                        bc[:, off : off + sz], pb[:],
                        mybir.ActivationFunctionType.Sigmoid,
                    )
                for cc in range(NCC):
                    nc.gpsimd.dma_start(out_v[b, cc], bc[:])

    nc.compile()
    return nc


def _prep_in_maps(x, w1, w2, wA, wB):
    x = np.ascontiguousarray(np.asarray(x, dtype=np.float32))
    w1 = np.asarray(w1, dtype=np.float32)
    w2 = np.asarray(w2, dtype=np.float32)
    wA = np.asarray(wA, dtype=np.float32)
    wB = np.asarray(wB, dtype=np.float32)

    w1t = np.ascontiguousarray(w1.T)                       # (C, HIDDEN)
    w2t = np.ascontiguousarray(w2.T)                       # (HIDDEN, C)
    # permute wA rows i*8+r -> r*56+i, then transpose: wat[c, r*56+i]
    wat = np.ascontiguousarray(
        wA.reshape(H, RANK, C).transpose(1, 0, 2).reshape(HR, C).T
    )
    wbt = np.ascontiguousarray(wB.T)                       # (C, HR), col r*56+j

    xs = x.reshape(NCORES, BLOC, C, HW)
    return [
        {"x": xs[i], "w1t": w1t, "w2t": w2t, "wat": wat, "wbt": wbt}
        for i in range(NCORES)
    ]


_NC_CACHE = None


def _get_nc():
    global _NC_CACHE
    if _NC_CACHE is None:
        _NC_CACHE = build_bass()
    return _NC_CACHE


def run(inputs: dict, trace: bool = False):
    """Run on 8 NeuronCores. Returns (full_output, BassKernelResults)."""
    in_maps = _prep_in_maps(**inputs)
    nc = _get_nc()
    res = run_bass_kernel_spmd(
        nc, in_maps, core_ids=list(range(NCORES)), trace=trace
    )
    out = np.stack([res.results[i]["out"] for i in range(NCORES)])
    return out.reshape(B, C, H, W).astype(np.float32, copy=False), res


def kernel(x, w1, w2, wA, wB):
    out, _ = run({"x": x, "w1": w1, "w2": w2, "wA": wA, "wB": wB})
    return out


# revision 8
# speedup vs baseline: 1.2270x; 1.2270x over previous
"""AdaptiveBlock Trainium2 kernel, 8-core data-parallel.

Reference computation (per batch b):
    y      = mean(x[b], axis=(H, W))                    # (C,)
    h      = gelu(y @ w1.T)                             # (HIDDEN,)
    y'     = gelu(h @ w2.T)                             # (C,)
    A      = (y' @ wA.T).reshape(H, R)
    Bm     = (y' @ wB.T).reshape(R, W)
    attn   = sigmoid(A @ Bm)                            # (H, W)
    out[b] = broadcast attn over C                      # (C, H, W)

Sharding: B=32 split over 8 cores (4 batches/core), weights replicated.
Memory-bound: each core reads 12.8 MB of x and writes 12.8 MB of output.

Device layout notes:
- y is computed transposed (C on partitions) directly by free-dim reduces.
- All weights are passed pre-transposed (contraction dim on partitions).
- wA additionally has its output columns permuted host-side from i*8+r to
  r*56+i so that the per-batch (8, 56) lhsT of the bilinear matmul can be
  sliced out of contiguous SBUF rows with a cheap SBUF->SBUF DMA.
- The (56, 56) attention map is flattened to (1, 3136), broadcast to all
  128 partitions with a K=1 ones-matmul on the TensorEngine, and sigmoid
  is fused into the PSUM->SBUF copy on the ScalarEngine.
"""

import numpy as np

import concourse.bass as bass
import concourse.tile as tile
from concourse import bacc
from concourse import mybir
from concourse.bass_utils import run_bass_kernel_spmd

F32 = mybir.dt.float32
BF16 = mybir.dt.bfloat16

B, C, H, W = 32, 256, 56, 56
HW = H * W                      # 3136
HIDDEN = 512
RANK = 8
HR = H * RANK                   # 448
NCORES = 8
BLOC = B // NCORES              # 4 batches per core
P = 128
NCC = C // P                    # 2 channel chunks
NHH = HIDDEN // P               # 4 hidden chunks
BCHUNK = 512                    # matmul moving free-dim max


def build_bass(sim_compat: bool = False) -> bacc.Bacc:
    """sim_compat=True swaps exact Gelu (not implemented in CoreSim) for a
    0.5*x stand-in; with |gelu-input| ~ 0.02 this perturbs the final sigmoid
    output by ~1e-5 relative, so the sim still validates all layout/dataflow.
    Hardware builds always use the exact erf-based Gelu."""
    gelu_f = (
        mybir.ActivationFunctionType.Copy
        if sim_compat
        else mybir.ActivationFunctionType.Gelu
    )
    gelu_s = 0.5 if sim_compat else 1.0
    nc = bacc.Bacc(num_devices=NCORES)

    x_d = nc.dram_tensor("x", [BLOC, C, HW], F32, kind="ExternalInput")
    w1t_d = nc.dram_tensor("w1t", [C, HIDDEN], F32, kind="ExternalInput")
    w2t_d = nc.dram_tensor("w2t", [HIDDEN, C], F32, kind="ExternalInput")
    wat_d = nc.dram_tensor("wat", [C, HR], F32, kind="ExternalInput")
    wbt_d = nc.dram_tensor("wbt", [C, HR], F32, kind="ExternalInput")
    out_d = nc.dram_tensor("out", [BLOC, C, HW], F32, kind="ExternalOutput")

    x_v = x_d.ap().rearrange("b (cc p) hw -> b cc p hw", p=P)
    out_v = out_d.ap().rearrange("b (cc p) hw -> b cc p hw", p=P)
    w1t_v = w1t_d.ap().rearrange("(cc p) h -> cc p h", p=P)
    w2t_v = w2t_d.ap().rearrange("(hh p) c -> hh p c", p=P)
    wat_v = wat_d.ap().rearrange("(cc p) n -> cc p n", p=P)
    wbt_v = wbt_d.ap().rearrange("(cc p) n -> cc p n", p=P)

    with tile.TileContext(nc) as tc:
        with (
            tc.tile_pool(name="xin", bufs=4) as xpool,
            tc.tile_pool(name="persist", bufs=1) as ppool,
            tc.tile_pool(name="small", bufs=2) as spool,
            tc.tile_pool(name="bc", bufs=4) as bcpool,
            tc.tile_pool(name="ps_small", bufs=3, space="PSUM") as ps_small,
            tc.tile_pool(name="ps_bc", bufs=4, space="PSUM") as ps_bc,
        ):
            # --- replicated weights into SBUF (contraction dim on partitions)
            w1t = [ppool.tile([P, HIDDEN], F32, tag=f"w1t{cc}", name=f"w1t{cc}") for cc in range(NCC)]
            w2t = [ppool.tile([P, C], F32, tag=f"w2t{hh}", name=f"w2t{hh}") for hh in range(NHH)]
            wat = [ppool.tile([P, HR], F32, tag=f"wat{cc}", name=f"wat{cc}") for cc in range(NCC)]
            wbt = [ppool.tile([P, HR], F32, tag=f"wbt{cc}", name=f"wbt{cc}") for cc in range(NCC)]
            for cc in range(NCC):
                nc.sync.dma_start(w1t[cc][:], w1t_v[cc])
                nc.sync.dma_start(wat[cc][:], wat_v[cc])
                nc.sync.dma_start(wbt[cc][:], wbt_v[cc])
            for hh in range(NHH):
                nc.sync.dma_start(w2t[hh][:], w2t_v[hh])
            ones = ppool.tile([1, P], BF16, tag="ones", name="ones")
            nc.vector.memset(ones[:], 1.0)

            # --- stream x in, spatial-sum each (b, channel-chunk) row
            ysum = [ppool.tile([P, BLOC], F32, tag=f"ysum{cc}", name=f"ysum{cc}") for cc in range(NCC)]
            for b in range(BLOC):
                for cc in range(NCC):
                    xt = xpool.tile([P, HW], F32, tag="xt", name="xt")
                    nc.gpsimd.dma_start(xt[:], x_v[b, cc])
                    nc.vector.reduce_sum(
                        ysum[cc][:, b : b + 1], xt[:], axis=mybir.AxisListType.X
                    )

            # --- MLP layer 1: hT[hh] = gelu((y @ w1.T).T chunk), y = ysum/HW
            hT = [ppool.tile([P, BLOC], F32, tag=f"hT{hh}", name=f"hT{hh}") for hh in range(NHH)]
            for hh in range(NHH):
                ph = ps_small.tile([P, BLOC], F32, tag="ps", name="ps")
                for cc in range(NCC):
                    nc.tensor.matmul(
                        ph[:],
                        w1t[cc][:, hh * P : (hh + 1) * P],
                        ysum[cc][:],
                        start=(cc == 0),
                        stop=(cc == NCC - 1),
                    )
                nc.scalar.activation(
                    hT[hh][:], ph[:], gelu_f, scale=gelu_s / HW,
                )

            # --- MLP layer 2: ypT[cc] = gelu((h @ w2.T).T chunk)
            ypT = [ppool.tile([P, BLOC], F32, tag=f"ypT{cc}", name=f"ypT{cc}") for cc in range(NCC)]
            for cc in range(NCC):
                pyp = ps_small.tile([P, BLOC], F32, tag="ps", name="ps")
                for hh in range(NHH):
                    nc.tensor.matmul(
                        pyp[:],
                        w2t[hh][:, cc * P : (cc + 1) * P],
                        hT[hh][:],
                        start=(hh == 0),
                        stop=(hh == NHH - 1),
                    )
                nc.scalar.activation(ypT[cc][:], pyp[:], gelu_f, scale=gelu_s)

            # --- A = y' @ waT (cols r*56+i), Bm = y' @ wbT (cols r*56+j)
            a_sb = ppool.tile([BLOC, HR], F32, tag="a_sb", name="a_sb")
            b_sb = ppool.tile([BLOC, HR], F32, tag="b_sb", name="b_sb")
            for dst, wt in ((a_sb, wat), (b_sb, wbt)):
                pab = ps_small.tile([BLOC, HR], F32, tag="ps", name="ps")
                for cc in range(NCC):
                    nc.tensor.matmul(
                        pab[:],
                        ypT[cc][:],
                        wt[cc][:],
                        start=(cc == 0),
                        stop=(cc == NCC - 1),
                    )
                nc.vector.tensor_copy(dst[:], pab[:])

            # --- per batch: bilinear, sigmoid, broadcast, store
            nchunks = (HW + BCHUNK - 1) // BCHUNK
            for b in range(BLOC):
                ar = spool.tile([RANK, H], F32, tag="ar", name="ar")
                br = spool.tile([RANK, W], F32, tag="br", name="br")
                nc.scalar.dma_start(
                    ar[:], a_sb[b : b + 1, :].rearrange("o (r i) -> o r i", r=RANK)
                )
                nc.scalar.dma_start(
                    br[:], b_sb[b : b + 1, :].rearrange("o (r j) -> o r j", r=RANK)
                )
                pm = ps_small.tile([H, W], F32, tag="ps", name="ps")
                nc.tensor.matmul(pm[:], ar[:], br[:], start=True, stop=True)
                msb = spool.tile([H, W], F32, tag="msb", name="msb")
                nc.scalar.copy(msb[:], pm[:])
                flat = spool.tile([1, HW], BF16, tag="flat", name="flat")
                nc.gpsimd.dma_start(
                    flat[0:1, :].rearrange("o (i j) -> o i j", i=H), msb[:, :]
                )
                bc = bcpool.tile([P, HW], F32, tag="bc", name="bc")
                for k in range(nchunks):
                    off = k * BCHUNK
                    sz = min(BCHUNK, HW - off)
                    pb = ps_bc.tile([P, sz], F32, tag="psbc", name="psbc")
                    nc.tensor.matmul(
                        pb[:], ones[:], flat[0:1, off : off + sz],
                        start=True, stop=True,
                    )
                    nc.scalar.activation(
                        bc[:, off : off + sz], pb[:],
                        mybir.ActivationFunctionType.Sigmoid,
                    )
                for cc in range(NCC):
                    nc.sync.dma_start(out_v[b, cc], bc[:])

    nc.compile()
    return nc


def _prep_in_maps(x, w1, w2, wA, wB):
    x = np.ascontiguousarray(np.asarray(x, dtype=np.float32))
    w1 = np.asarray(w1, dtype=np.float32)
    w2 = np.asarray(w2, dtype=np.float32)
    wA = np.asarray(wA, dtype=np.float32)
    wB = np.asarray(wB, dtype=np.float32)

    w1t = np.ascontiguousarray(w1.T)                       # (C, HIDDEN)
    w2t = np.ascontiguousarray(w2.T)                       # (HIDDEN, C)
    # permute wA rows i*8+r -> r*56+i, then transpose: wat[c, r*56+i]
    wat = np.ascontiguousarray(
        wA.reshape(H, RANK, C).transpose(1, 0, 2).reshape(HR, C).T
    )
    wbt = np.ascontiguousarray(wB.T)                       # (C, HR), col r*56+j

    xs = x.reshape(NCORES, BLOC, C, HW)
    return [
        {"x": xs[i], "w1t": w1t, "w2t": w2t, "wat": wat, "wbt": wbt}
        for i in range(NCORES)
    ]


_NC_CACHE = None


def _get_nc():
    global _NC_CACHE
    if _NC_CACHE is None:
        _NC_CACHE = build_bass()
    return _NC_CACHE


def run(inputs: dict, trace: bool = False):
    """Run on 8 NeuronCores. Returns (full_output, BassKernelResults)."""
    in_maps = _prep_in_maps(**inputs)
    nc = _get_nc()
    res = run_bass_kernel_spmd(
        nc, in_maps, core_ids=list(range(NCORES)), trace=trace
    )
    out = np.stack([res.results[i]["out"] for i in range(NCORES)])
    return out.reshape(B, C, H, W).astype(np.float32, copy=False), res


def kernel(x, w1, w2, wA, wB):
    out, _ = run({"x": x, "w1": w1, "w2": w2, "wA": wA, "wB": wB})
    return out


# revision 9
# speedup vs baseline: 1.4536x; 1.1847x over previous
"""AdaptiveBlock Trainium2 kernel, 8-core data-parallel.

Reference computation (per batch b):
    y      = mean(x[b], axis=(H, W))                    # (C,)
    h      = gelu(y @ w1.T)                             # (HIDDEN,)
    y'     = gelu(h @ w2.T)                             # (C,)
    A      = (y' @ wA.T).reshape(H, R)
    Bm     = (y' @ wB.T).reshape(R, W)
    attn   = sigmoid(A @ Bm)                            # (H, W)
    out[b] = broadcast attn over C                      # (C, H, W)

Sharding: B=32 split over 8 cores (4 batches/core), weights replicated.
Memory-bound: each core reads 12.8 MB of x and writes 12.8 MB of output;
HBM roofline ~72 us/core.

Device layout / scheduling notes:
- y is computed transposed (C on partitions) directly by free-dim reduces.
- All weights are passed pre-transposed AND pre-cast to bf16 host-side
  (contraction dim on partitions); matmul operands are bf16, PSUM f32.
  The f32 sums feeding the MLP are cast to bf16 on the ScalarEngine.
  All activations here are ~1e-2 magnitude and the output is sigmoid
  (~0.5 +- 1e-4), so bf16 operand noise is ~1e-6 relative on the output.
- wA additionally has its output columns permuted host-side from i*8+r to
  r*56+i so the per-batch (8, 56) bilinear lhsT is a contiguous-row
  SBUF->SBUF DMA.
- The (56, 56) attention map is flattened to (1, 3136) bf16, broadcast to
  128 partitions by a K=1 ones-matmul on the TensorEngine, with sigmoid
  fused into the PSUM->SBUF copy on the ScalarEngine.
- The 4 batches are processed in 2 groups of 2 so group 0's output DMAs
  overlap group 1's input DMAs. Inputs ride the SWDGE (gpsimd) queue,
  outputs + weights the SP HWDGE queue, small SBUF->SBUF moves the ACT
  HWDGE queue - three independent DMA dispatch FIFOs.
"""

import numpy as np
import ml_dtypes

import concourse.bass as bass
import concourse.tile as tile
from concourse import bacc, mybir
from concourse.bass_utils import run_bass_kernel_spmd

F32 = mybir.dt.float32
BF16 = mybir.dt.bfloat16

B, C, H, W = 32, 256, 56, 56
HW = H * W                      # 3136
HIDDEN = 512
RANK = 8
HR = H * RANK                   # 448
NCORES = 8
BLOC = B // NCORES              # 4 batches per core
P = 128
NCC = C // P                    # 2 channel chunks
NHH = HIDDEN // P               # 4 hidden chunks
BCHUNK = 512                    # matmul moving free-dim max (PSUM bank)
NGRP = 2                        # batch groups per core
GB = BLOC // NGRP               # batches per group


def build_bass(sim_compat: bool = False) -> bacc.Bacc:
    """sim_compat=True swaps exact Gelu (not implemented in CoreSim) for a
    0.5*x stand-in; with |gelu-input| ~ 0.02 this perturbs the final sigmoid
    output by ~1e-5 relative, so the sim still validates all layout/dataflow.
    Hardware builds always use the exact erf-based Gelu."""
    gelu_f = (
        mybir.ActivationFunctionType.Copy
        if sim_compat
        else mybir.ActivationFunctionType.Gelu
    )
    gelu_s = 0.5 if sim_compat else 1.0
    nc = bacc.Bacc(num_devices=NCORES)

    x_d = nc.dram_tensor("x", [BLOC, C, HW], F32, kind="ExternalInput")
    w1t_d = nc.dram_tensor("w1t", [C, HIDDEN], BF16, kind="ExternalInput")
    w2t_d = nc.dram_tensor("w2t", [HIDDEN, C], BF16, kind="ExternalInput")
    wat_d = nc.dram_tensor("wat", [C, HR], BF16, kind="ExternalInput")
    wbt_d = nc.dram_tensor("wbt", [C, HR], BF16, kind="ExternalInput")
    out_d = nc.dram_tensor("out", [BLOC, C, HW], F32, kind="ExternalOutput")

    x_v = x_d.ap().rearrange("b (cc p) hw -> b cc p hw", p=P)
    out_v = out_d.ap().rearrange("b (cc p) hw -> b cc p hw", p=P)
    w1t_v = w1t_d.ap().rearrange("(cc p) h -> cc p h", p=P)
    w2t_v = w2t_d.ap().rearrange("(hh p) c -> hh p c", p=P)
    wat_v = wat_d.ap().rearrange("(cc p) n -> cc p n", p=P)
    wbt_v = wbt_d.ap().rearrange("(cc p) n -> cc p n", p=P)

    nchunks = (HW + BCHUNK - 1) // BCHUNK

    with tile.TileContext(nc) as tc:
        with (
            tc.tile_pool(name="xin", bufs=4) as xpool,
            tc.tile_pool(name="persist", bufs=1) as ppool,
            tc.tile_pool(name="small", bufs=2) as spool,
            tc.tile_pool(name="bc", bufs=4) as bcpool,
            tc.tile_pool(name="ps_small", bufs=3, space="PSUM") as ps_small,
            tc.tile_pool(name="ps_bc", bufs=4, space="PSUM") as ps_bc,
        ):
            # --- replicated bf16 weights (contraction dim on partitions)
            w1t = [ppool.tile([P, HIDDEN], BF16, tag=f"w1t{cc}", name=f"w1t{cc}")
                   for cc in range(NCC)]
            w2t = [ppool.tile([P, C], BF16, tag=f"w2t{hh}", name=f"w2t{hh}")
                   for hh in range(NHH)]
            wat = [ppool.tile([P, HR], BF16, tag=f"wat{cc}", name=f"wat{cc}")
                   for cc in range(NCC)]
            wbt = [ppool.tile([P, HR], BF16, tag=f"wbt{cc}", name=f"wbt{cc}")
                   for cc in range(NCC)]
            for cc in range(NCC):
                nc.sync.dma_start(w1t[cc][:], w1t_v[cc])
                nc.sync.dma_start(wat[cc][:], wat_v[cc])
                nc.sync.dma_start(wbt[cc][:], wbt_v[cc])
            for hh in range(NHH):
                nc.sync.dma_start(w2t[hh][:], w2t_v[hh])
            ones = ppool.tile([1, P], BF16, tag="ones", name="ones")
            nc.vector.memset(ones[:], 1.0)

            for g in range(NGRP):
                # --- stream this group's x in, spatial-sum per (b, chunk)
                ysum = [ppool.tile([P, GB], F32, tag=f"ysum{g}{cc}",
                                   name=f"ysum{g}{cc}") for cc in range(NCC)]
                for j in range(GB):
                    b = g * GB + j
                    for cc in range(NCC):
                        xt = xpool.tile([P, HW], F32, tag="xt", name="xt")
                        nc.gpsimd.dma_start(xt[:], x_v[b, cc])
                        nc.vector.reduce_sum(
                            ysum[cc][:, j : j + 1], xt[:],
                            axis=mybir.AxisListType.X,
                        )
                ysb = [ppool.tile([P, GB], BF16, tag=f"ysb{g}{cc}",
                                  name=f"ysb{g}{cc}") for cc in range(NCC)]
                for cc in range(NCC):
                    nc.scalar.copy(ysb[cc][:], ysum[cc][:])

                # --- MLP layer 1: hT[hh] = gelu((y @ w1.T).T), y = ysum/HW
                hT = [ppool.tile([P, GB], BF16, tag=f"hT{g}{hh}",
                                 name=f"hT{g}{hh}") for hh in range(NHH)]
                for hh in range(NHH):
                    ph = ps_small.tile([P, GB], F32, tag="ps", name="ps")
                    for cc in range(NCC):
                        nc.tensor.matmul(
                            ph[:], w1t[cc][:, hh * P : (hh + 1) * P], ysb[cc][:],
                            start=(cc == 0), stop=(cc == NCC - 1),
                        )
                    nc.scalar.activation(hT[hh][:], ph[:], gelu_f,
                                         scale=gelu_s / HW)

                # --- MLP layer 2: ypT[cc] = gelu((h @ w2.T).T)
                ypT = [ppool.tile([P, GB], BF16, tag=f"ypT{g}{cc}",
                                  name=f"ypT{g}{cc}") for cc in range(NCC)]
                for cc in range(NCC):
                    pyp = ps_small.tile([P, GB], F32, tag="ps", name="ps")
                    for hh in range(NHH):
                        nc.tensor.matmul(
                            pyp[:], w2t[hh][:, cc * P : (cc + 1) * P], hT[hh][:],
                            start=(hh == 0), stop=(hh == NHH - 1),
                        )
                    nc.scalar.activation(ypT[cc][:], pyp[:], gelu_f, scale=gelu_s)

                # --- A = y' @ waT (cols r*56+i), Bm = y' @ wbT (cols r*56+j)
                a_sb = ppool.tile([GB, HR], BF16, tag=f"a_sb{g}", name=f"a_sb{g}")
                b_sb = ppool.tile([GB, HR], BF16, tag=f"b_sb{g}", name=f"b_sb{g}")
                for dst, wt in ((a_sb, wat), (b_sb, wbt)):
                    pab = ps_small.tile([GB, HR], F32, tag="ps", name="ps")
                    for cc in range(NCC):
                        nc.tensor.matmul(
                            pab[:], ypT[cc][:], wt[cc][:],
                            start=(cc == 0), stop=(cc == NCC - 1),
                        )
                    nc.vector.tensor_copy(dst[:], pab[:])

                # --- per batch: bilinear, broadcast, sigmoid, store
                for j in range(GB):
                    b = g * GB + j
                    ar = spool.tile([RANK, H], BF16, tag="ar", name="ar")
                    br = spool.tile([RANK, W], BF16, tag="br", name="br")
                    nc.scalar.dma_start(
                        ar[:],
                        a_sb[j : j + 1, :].rearrange("o (r i) -> o r i", r=RANK),
                    )
                    nc.scalar.dma_start(
                        br[:],
                        b_sb[j : j + 1, :].rearrange("o (r j) -> o r j", r=RANK),
                    )
                    pm = ps_small.tile([H, W], F32, tag="ps", name="ps")
                    nc.tensor.matmul(pm[:], ar[:], br[:], start=True, stop=True)
                    msb = spool.tile([H, W], BF16, tag="msb", name="msb")
                    nc.scalar.copy(msb[:], pm[:])
                    flat = spool.tile([1, HW], BF16, tag="flat", name="flat")
                    nc.scalar.dma_start(
                        flat[0:1, :].rearrange("o (i j) -> o i j", i=H), msb[:, :]
                    )
                    bc = bcpool.tile([P, HW], F32, tag="bc", name="bc")
                    for k in range(nchunks):
                        off = k * BCHUNK
                        sz = min(BCHUNK, HW - off)
                        pb = ps_bc.tile([P, sz], F32, tag="psbc", name="psbc")
                        nc.tensor.matmul(
                            pb[:], ones[:], flat[0:1, off : off + sz],
                            start=True, stop=True,
                        )
                        nc.scalar.activation(
                            bc[:, off : off + sz], pb[:],
                            mybir.ActivationFunctionType.Sigmoid,
                        )
                    for cc in range(NCC):
                        nc.sync.dma_start(out_v[b, cc], bc[:])

    nc.compile()
    return nc


def _prep_in_maps(x, w1, w2, wA, wB):
    x = np.ascontiguousarray(np.asarray(x, dtype=np.float32))
    w1 = np.asarray(w1, dtype=np.float32)
    w2 = np.asarray(w2, dtype=np.float32)
    wA = np.asarray(wA, dtype=np.float32)
    wB = np.asarray(wB, dtype=np.float32)

    bf = ml_dtypes.bfloat16
    w1t = np.ascontiguousarray(w1.T).astype(bf)            # (C, HIDDEN)
    w2t = np.ascontiguousarray(w2.T).astype(bf)            # (HIDDEN, C)
    # permute wA rows i*8+r -> r*56+i, then transpose: wat[c, r*56+i]
    wat = np.ascontiguousarray(
        wA.reshape(H, RANK, C).transpose(1, 0, 2).reshape(HR, C).T
    ).astype(bf)
    wbt = np.ascontiguousarray(wB.T).astype(bf)            # (C, HR), col r*56+j

    xs = x.reshape(NCORES, BLOC, C, HW)
    return [
        {"x": xs[i], "w1t": w1t, "w2t": w2t, "wat": wat, "wbt": wbt}
        for i in range(NCORES)
    ]


_NC_CACHE = None


def _get_nc():
    global _NC_CACHE
    if _NC_CACHE is None:
        _NC_CACHE = build_bass()
    return _NC_CACHE


def run(inputs: dict, trace: bool = False):
    """Run on 8 NeuronCores. Returns (full_output, BassKernelResults)."""
    in_maps = _prep_in_maps(**inputs)
    nc = _get_nc()
    res = run_bass_kernel_spmd(
        nc, in_maps, core_ids=list(range(NCORES)), trace=trace
    )
    out = np.stack([res.results[i]["out"] for i in range(NCORES)])
    return out.reshape(B, C, H, W).astype(np.float32, copy=False), res


def kernel(x, w1, w2, wA, wB):
    out, _ = run({"x": x, "w1": w1, "w2": w2, "wA": wA, "wB": wB})
    return out
